# revision 9
# baseline (speedup 1.0000x reference)
"""Trainium2 Bass kernel for out = x @ expm(skew(angles)) + bias.

Strategy:
  - Data-parallel over the batch: x [16384, 512] is split into 8 shards of
    [2048, 512], one per NeuronCore. angles/bias are replicated.
  - Host only does layout: builds A = skew(angles) in bf16 and ships each
    core its x shard pre-transposed ([512, 2048]) in bf16 so the
    contraction dim lands on SBUF partitions. All linear algebra runs
    on-device. The output comes back transposed ([512, 2048] per core)
    and is un-transposed on host -- this puts the output *column* on the
    SBUF partition axis, which makes the bias a per-partition scalar that
    the scalar engine can add (splitting the PSUM-drain work across two
    engines; gpsimd cannot touch PSUM).
  - Rotation W = expm(A) via a degree-4 Taylor series in
    Paterson-Stockmeyer form (2 matmuls of 512^3), exploiting
    skew-symmetry so no transposes or negations are ever materialized:
        nA2 = A^T @ A              (= -A^2;  lhsT = rhs = A directly)
        A6  = -nA2 / 6             (scalar-engine scaled copy = A^2/6)
        Y   = A - nA2/4            (one DVE op per tile, = A + A^2/4)
        T   = (I + A) - nA2/2      (I built on-device via affine_select)
        F   = A6 @ Y               (A6 symmetric => its own lhsT; = A^3/6 + A^4/24)
        W   = T + F
    Truncation error ||A||^5/5! ~ 2e-4 for ||A||_2 ~ 0.48, far below the
    2e-2 gate.
  - Dtypes: all matmul operands are bf16 (the backend forbids mixing
    32-bit and 16-bit operands), streaming 1 PE cycle per output row --
    4x faster than fp32 two-pass mode -- and halving the x DMA. PSUM
    accumulation is fp32; the output is exact fp32. Measured end-to-end
    max rel err ~4e-3, dominated by the bf16 rounding of x and W.
  - Pipeline scheduling (from trace analysis): mm1 runs i-major so its 4
    PSUM groups stop staggered ~0.9us apart; the scaled copy (scalar) and
    Y/T (vector) for group i start as soon as group i stops, letting
    t-major mm2 start the moment mm1's last matmul retires. W is
    finished in 128-column quarters, emitted in the exact order the main
    loop consumes them, so the main loop starts right behind mm2.
  - PSUM budget: mm1 + main loop rotate 4 banks (eps pool), mm2 uses the
    other 4 (ops pool), so no engine ever stalls on a bank recycle.
  - Main loop: psum group (nt, mc) = out.T[128-col block nt, 512-row
    chunk mc] accumulates 4 matmuls of N=512; the PSUM->SBUF bias-add
    alternates scalar/vector per group.
"""

import numpy as np
import ml_dtypes

import concourse.bacc as bacc
import concourse.bass as bass
import concourse.mybir as mybir
import concourse.tile as tile
from concourse.bass_utils import run_bass_kernel_spmd

DIM = 512
BATCH = 16384
N_CORES = 8
XB = BATCH // N_CORES          # rows per core
P = 128                        # partitions
KT = DIM // P                  # 4 k-tiles
MT = XB // P                   # 16 m-tiles per core
XC = 4                         # m-tiles per x DMA chunk
F32 = mybir.dt.float32
BF16 = mybir.dt.bfloat16

_CACHE = {}


def build_bass():
    nc = bacc.Bacc("TRN2", target_bir_lowering=False, debug=False)

    xt_d = nc.dram_tensor("xt", [DIM, XB], BF16, kind="ExternalInput")
    ar_d = nc.dram_tensor("ar", [DIM, DIM], BF16, kind="ExternalInput")
    biasq_d = nc.dram_tensor("biasq", [P, KT], F32, kind="ExternalInput")
    out_d = nc.dram_tensor("out", [DIM, XB], F32, kind="ExternalOutput")

    AOP = mybir.AluOpType

    with tile.TileContext(nc) as tc:
        with (
            tc.tile_pool(name="const", bufs=1) as cpool,
            tc.tile_pool(name="xin", bufs=MT // XC) as xpool,
            tc.tile_pool(name="oout", bufs=4) as opool,
            tc.tile_pool(name="eps", bufs=4, space=bass.MemorySpace.PSUM) as eps,
            tc.tile_pool(name="ops", bufs=4, space=bass.MemorySpace.PSUM) as ops,
        ):
            ar_sb = cpool.tile([P, KT, DIM], BF16)     # A
            ai_sb = cpool.tile([P, KT, DIM], BF16)     # A + I
            a6_sb = cpool.tile([P, KT, DIM], BF16)     # A^2/6 (mm2 lhsT)
            y_sb = cpool.tile([P, KT, DIM], BF16)      # Y = A + A^2/4
            t_sb = cpool.tile([P, KT, DIM], F32)       # T = (I+A) + A^2/2
            m_sb = cpool.tile([P, KT, DIM], BF16)      # W
            biasq_sb = cpool.tile([P, KT], F32)

            # ---- input DMAs, priority order: A tiles gate everything ----
            for t in range(KT):
                nc.sync.dma_start(ar_sb[:, t, :], ar_d[P * t : P * (t + 1), :])
            xch = []
            for c in range(MT // XC):
                xc = xpool.tile([P, KT, P * XC], BF16, tag="x")
                nc.sync.dma_start(
                    xc[:, :, :],
                    xt_d[:, P * XC * c : P * XC * (c + 1)].rearrange(
                        "(t p) m -> p t m", p=P
                    ),
                )
                xch.append(xc)
            nc.sync.dma_start(biasq_sb[:, :], biasq_d[:, :])

            # ---- ai = A with exact 1.0 on the diagonal (gpsimd, paced
            # only by the A-tile DMAs; global row P*t + p == col n) ----
            for t in range(KT):
                nc.gpsimd.affine_select(
                    out=ai_sb[:, t, :],
                    in_=ar_sb[:, t, :],
                    compare_op=AOP.not_equal,
                    fill=1.0,
                    base=-P * t,
                    channel_multiplier=-1,
                    pattern=[[1, DIM]],
                )

            # ---- mm1: nA2 = A^T @ A = -A^2, i-major so the 4 PSUM
            # groups stop staggered and downstream work starts early ----
            pss = [eps.tile([P, DIM], F32, tag="eps", name=f"pss{i}") for i in range(KT)]
            for i in range(KT):
                for t in range(KT):
                    nc.tensor.matmul(
                        pss[i][:, :],
                        ar_sb[:, t, P * i : P * (i + 1)],
                        ar_sb[:, t, :],
                        start=(t == 0),
                        stop=(t == KT - 1),
                    )
            # scalar engine: A6 = A^2/6 bf16 copy (the mm2 lhsT)
            for i in range(KT):
                nc.scalar.mul(a6_sb[:, i, :], pss[i][:, :], -1.0 / 6.0)
            # vector: Y_i then T_i right behind each group's stop
            for i in range(KT):
                nc.vector.scalar_tensor_tensor(
                    y_sb[:, i, :], pss[i][:, :], -0.25, ar_sb[:, i, :],
                    AOP.mult, AOP.add,
                )
                nc.vector.scalar_tensor_tensor(
                    t_sb[:, i, :], pss[i][:, :], -0.5, ai_sb[:, i, :],
                    AOP.mult, AOP.add,
                )

            # ---- mm2: F = A6 @ Y = A^3/6 + A^4/24, t-major (its deps
            # arrive in t order; starts the moment mm1 retires) ----
            ps2 = [ops.tile([P, DIM], F32, tag="m2", name=f"ps2{i}") for i in range(KT)]
            for t in range(KT):
                for i in range(KT):
                    nc.tensor.matmul(
                        ps2[i][:, :],
                        a6_sb[:, t, P * i : P * (i + 1)],
                        y_sb[:, t, :],
                        start=(t == 0),
                        stop=(t == KT - 1),
                    )
            # W = T + F in 128-col quarters, in main-loop consumption
            # order (q = the nt block the main loop reads as lhsT)
            for q in range(KT):
                for i in range(KT):
                    nc.vector.tensor_add(
                        m_sb[:, i, P * q : P * (q + 1)],
                        ps2[i][:, P * q : P * (q + 1)],
                        t_sb[:, i, P * q : P * (q + 1)],
                    )

            # ---- main loop: out.T = W.T @ x.T + bias ----
            # group (nt, mc): out.T rows [128*nt, 128*(nt+1)), batch cols
            # [512*mc, 512*(mc+1)); bias-add alternates scalar/vector
            for nt in range(KT):
                for mc in range(MT // XC):
                    xc = xch[mc]
                    ps = eps.tile([P, DIM], F32, tag="eps", name=f"mo{nt}_{mc}")
                    for kb in range(KT):
                        nc.tensor.matmul(
                            ps[:, :],
                            m_sb[:, kb, P * nt : P * (nt + 1)],
                            xc[:, kb, :],
                            start=(kb == 0),
                            stop=(kb == KT - 1),
                        )
                    ot = opool.tile([P, P * XC], F32, tag="o")
                    if mc % 2 == 0:
                        nc.scalar.add(ot[:, :], ps[:, :], biasq_sb[:, nt : nt + 1])
                    else:
                        nc.vector.tensor_scalar_add(
                            ot[:, :], ps[:, :], biasq_sb[:, nt : nt + 1]
                        )
                    nc.sync.dma_start(
                        out_d[P * nt : P * (nt + 1), P * XC * mc : P * XC * (mc + 1)],
                        ot[:, :],
                    )

    nc.compile()
    return nc


def _get_nc():
    if "nc" not in _CACHE:
        _CACHE["nc"] = build_bass()
    return _CACHE["nc"]


def _host_inputs(angles, bias):
    angles = np.asarray(angles, dtype=np.float32)
    bias = np.asarray(bias, dtype=np.float32)
    iu, ju = np.triu_indices(DIM, k=1)
    A = np.zeros((DIM, DIM), dtype=np.float32)
    A[iu, ju] = angles
    A[ju, iu] = -angles
    return {
        "ar": A.astype(ml_dtypes.bfloat16),
        # biasq[p, t] = bias[128*t + p]
        "biasq": np.ascontiguousarray(bias.reshape(KT, P).T),
    }


def kernel(x, angles, bias, _profile=False):
    x = np.asarray(x, dtype=np.float32)
    # per-core x shards, pre-transposed to [DIM, XB] bf16 (layout only)
    xts = np.ascontiguousarray(
        x.reshape(N_CORES, XB, DIM).transpose(0, 2, 1)
    ).astype(ml_dtypes.bfloat16)
    shared = _host_inputs(angles, bias)
    nc = _get_nc()
    in_maps = [{"xt": xts[c], **shared} for c in range(N_CORES)]
    res = run_bass_kernel_spmd(
        nc, in_maps, list(range(N_CORES)), trace=bool(_profile)
    )
    _CACHE["last_result"] = res
    # device returns out.T per core; un-transpose on host (layout only)
    out = np.concatenate(
        [np.ascontiguousarray(res.results[c]["out"].T) for c in range(N_CORES)],
        axis=0,
    )
    return out


# revision 14
# speedup vs baseline: 1.0532x; 1.0532x over previous
"""Trainium2 Bass kernel for out = x @ expm(skew(angles)) + bias.

Strategy:
  - Data-parallel over the batch: x [16384, 512] is split into 8 shards of
    [2048, 512], one per NeuronCore. angles/bias are replicated.
  - Host only does layout: builds A = skew(angles) in bf16 and ships each
    core its x shard pre-transposed ([512, 2048]) in bf16 so the
    contraction dim lands on SBUF partitions. All linear algebra runs
    on-device. The output comes back transposed ([512, 2048] per core)
    and is un-transposed on host -- this puts the output *column* on the
    SBUF partition axis, which makes the bias a per-partition scalar that
    the scalar engine can add (splitting the PSUM-drain work across two
    engines; gpsimd cannot touch PSUM).
  - Rotation W = expm(A) via a degree-4 Taylor series in
    Paterson-Stockmeyer form (2 matmuls of 512^3), exploiting
    skew-symmetry so no transposes or negations are ever materialized:
        nA2 = A^T @ A              (= -A^2;  lhsT = rhs = A directly)
        A6  = -nA2 / 6             (scalar-engine scaled copy = A^2/6)
        Y   = A - nA2/4            (one DVE op per tile, = A + A^2/4)
        T   = (I + A) - nA2/2      (I built on-device via affine_select)
        F   = A6 @ Y               (A6 symmetric => its own lhsT; = A^3/6 + A^4/24)
        W   = T + F
    Truncation error ||A||^5/5! ~ 2e-4 for ||A||_2 ~ 0.48, far below the
    2e-2 gate.
  - Dtypes: all matmul operands are bf16 (the backend forbids mixing
    32-bit and 16-bit operands), streaming 1 PE cycle per output row --
    4x faster than fp32 two-pass mode -- and halving the x DMA. PSUM
    accumulation is fp32; the output is exact fp32. Measured end-to-end
    max rel err ~4e-3, dominated by the bf16 rounding of x and W.
  - Pipeline scheduling (from trace analysis): mm1 runs i-major so its 4
    PSUM groups stop staggered ~0.9us apart; the scaled copy (scalar) and
    Y/T (vector) for group i start as soon as group i stops, letting
    t-major mm2 start the moment mm1's last matmul retires. W is
    finished in 128-column quarters, emitted in the exact order the main
    loop consumes them, so the main loop starts right behind mm2.
  - PSUM budget: mm1 + main loop rotate 4 banks (eps pool), mm2 uses the
    other 4 (ops pool), so no engine ever stalls on a bank recycle.
  - Main loop: psum group (nt, mc) = out.T[128-col block nt, 512-row
    chunk mc] accumulates 4 matmuls of N=512; the PSUM->SBUF bias-add
    alternates scalar/vector per group.
"""

import numpy as np
import ml_dtypes

import concourse.bacc as bacc
import concourse.bass as bass
import concourse.mybir as mybir
import concourse.tile as tile
from concourse.bass_utils import run_bass_kernel_spmd

DIM = 512
BATCH = 16384
N_CORES = 8
XB = BATCH // N_CORES          # rows per core
P = 128                        # partitions
KT = DIM // P                  # 4 k-tiles
MT = XB // P                   # 16 m-tiles per core
XC = 4                         # m-tiles per x DMA chunk
NWARM = 30                     # clock-ramp warmup matmuls (~107ns each)
F32 = mybir.dt.float32
BF16 = mybir.dt.bfloat16

_CACHE = {}


def build_bass():
    nc = bacc.Bacc("TRN2", target_bir_lowering=False, debug=False)

    xt_d = nc.dram_tensor("xt", [DIM, XB], BF16, kind="ExternalInput")
    ar_d = nc.dram_tensor("ar", [DIM, DIM], BF16, kind="ExternalInput")
    biasq_d = nc.dram_tensor("biasq", [P, KT], F32, kind="ExternalInput")
    out_d = nc.dram_tensor("out", [DIM, XB], F32, kind="ExternalOutput")

    AOP = mybir.AluOpType

    with tile.TileContext(nc) as tc:
        with (
            tc.tile_pool(name="const", bufs=1) as cpool,
            tc.tile_pool(name="xin", bufs=MT // XC) as xpool,
            tc.tile_pool(name="oout", bufs=6) as opool,
            tc.tile_pool(name="eps", bufs=4, space=bass.MemorySpace.PSUM) as eps,
            tc.tile_pool(name="ops", bufs=4, space=bass.MemorySpace.PSUM) as ops,
        ):
            ar_sb = cpool.tile([P, KT, DIM], BF16)     # A
            ai_sb = cpool.tile([P, KT, DIM], BF16)     # A + I
            a6_sb = cpool.tile([P, KT, DIM], BF16)     # A^2/6 (mm2 lhsT)
            y_sb = cpool.tile([P, KT, DIM], BF16)      # Y = A + A^2/4
            t_sb = cpool.tile([P, KT, DIM], F32)       # T = (I+A) + A^2/2
            m_sb = cpool.tile([P, KT, DIM], BF16)      # W
            biasq_sb = cpool.tile([P, KT], F32)
            warm_sb = cpool.tile([P, P], BF16)

            # ---- PE clock-ramp warmup: the PE p-state only ramps under
            # continuous load, and A always lands ~3-5us after the
            # prologue barrier (its DMA issue waits for the same
            # barrier), so a warmup burst bridges exactly that window ----
            nc.gpsimd.memset(warm_sb[:, :], 0.0)
            wp = ops.tile([P, DIM], F32, tag="m2", name="wp")
            for _ in range(NWARM):
                nc.tensor.matmul(
                    wp[:, 0:P], warm_sb[:, :], warm_sb[:, :], start=True, stop=True
                )

            # ---- input DMAs; the 4 A tiles issue in parallel from 4
            # engine queues at barrier exit (they gate everything) ----
            for t, eng in enumerate((nc.sync, nc.scalar, nc.gpsimd, nc.sync)):
                eng.dma_start(ar_sb[:, t, :], ar_d[P * t : P * (t + 1), :])
            xch = []
            for c in range(MT // XC):
                xc = xpool.tile([P, KT, P * XC], BF16, tag="x")
                nc.sync.dma_start(
                    xc[:, :, :],
                    xt_d[:, P * XC * c : P * XC * (c + 1)].rearrange(
                        "(t p) m -> p t m", p=P
                    ),
                )
                xch.append(xc)
            nc.sync.dma_start(biasq_sb[:, :], biasq_d[:, :])

            # ---- ai = A with exact 1.0 on the diagonal (gpsimd, paced
            # only by the A-tile DMAs; global row P*t + p == col n) ----
            for t in range(KT):
                nc.gpsimd.affine_select(
                    out=ai_sb[:, t, :],
                    in_=ar_sb[:, t, :],
                    compare_op=AOP.not_equal,
                    fill=1.0,
                    base=-P * t,
                    channel_multiplier=-1,
                    pattern=[[1, DIM]],
                )

            # ---- mm1: nA2 = A^T @ A = -A^2, i-major so the 4 PSUM
            # groups stop staggered and downstream work starts early ----
            pss = [eps.tile([P, DIM], F32, tag="eps", name=f"pss{i}") for i in range(KT)]
            for i in range(KT):
                for t in range(KT):
                    nc.tensor.matmul(
                        pss[i][:, :],
                        ar_sb[:, t, P * i : P * (i + 1)],
                        ar_sb[:, t, :],
                        start=(t == 0),
                        stop=(t == KT - 1),
                    )
            # scalar engine: A6 = A^2/6 bf16 copy (the mm2 lhsT)
            for i in range(KT):
                nc.scalar.mul(a6_sb[:, i, :], pss[i][:, :], -1.0 / 6.0)
            # vector: Y_i then T_i right behind each group's stop
            for i in range(KT):
                nc.vector.scalar_tensor_tensor(
                    y_sb[:, i, :], pss[i][:, :], -0.25, ar_sb[:, i, :],
                    AOP.mult, AOP.add,
                )
                nc.vector.scalar_tensor_tensor(
                    t_sb[:, i, :], pss[i][:, :], -0.5, ai_sb[:, i, :],
                    AOP.mult, AOP.add,
                )

            # ---- mm2: F = A6 @ Y = A^3/6 + A^4/24, t-major (its deps
            # arrive in t order; starts the moment mm1 retires) ----
            # ps2[3] reuses the warmup bank (ops pool rotation; the
            # warmups retire long before mm2 reaches t=0 for group 3)
            ps2 = [ops.tile([P, DIM], F32, tag="m2", name=f"ps2{i}") for i in range(KT)]
            for t in range(KT):
                for i in range(KT):
                    nc.tensor.matmul(
                        ps2[i][:, :],
                        a6_sb[:, t, P * i : P * (i + 1)],
                        y_sb[:, t, :],
                        start=(t == 0),
                        stop=(t == KT - 1),
                    )
            # W = T + F in 128-col quarters, in main-loop consumption
            # order (q = the nt block the main loop reads as lhsT)
            for q in range(KT):
                for i in range(KT):
                    nc.vector.tensor_add(
                        m_sb[:, i, P * q : P * (q + 1)],
                        ps2[i][:, P * q : P * (q + 1)],
                        t_sb[:, i, P * q : P * (q + 1)],
                    )

            # ---- main loop: out.T = W.T @ x.T + bias ----
            # group (nt, mc): out.T rows [128*nt, 128*(nt+1)), batch cols
            # [512*mc, 512*(mc+1)); bias-add alternates scalar/vector
            for nt in range(KT):
                for mc in range(MT // XC):
                    xc = xch[mc]
                    ps = eps.tile([P, DIM], F32, tag="eps", name=f"mo{nt}_{mc}")
                    for kb in range(KT):
                        nc.tensor.matmul(
                            ps[:, :],
                            m_sb[:, kb, P * nt : P * (nt + 1)],
                            xc[:, kb, :],
                            start=(kb == 0),
                            stop=(kb == KT - 1),
                        )
                    ot = opool.tile([P, P * XC], F32, tag="o")
                    if mc % 2 == 0:
                        nc.scalar.add(ot[:, :], ps[:, :], biasq_sb[:, nt : nt + 1])
                    else:
                        nc.vector.tensor_scalar_add(
                            ot[:, :], ps[:, :], biasq_sb[:, nt : nt + 1]
                        )
                    nc.sync.dma_start(
                        out_d[P * nt : P * (nt + 1), P * XC * mc : P * XC * (mc + 1)],
                        ot[:, :],
                    )

    nc.compile()
    return nc


def _get_nc():
    if "nc" not in _CACHE:
        _CACHE["nc"] = build_bass()
    return _CACHE["nc"]


def _host_inputs(angles, bias):
    angles = np.asarray(angles, dtype=np.float32)
    bias = np.asarray(bias, dtype=np.float32)
    iu, ju = np.triu_indices(DIM, k=1)
    A = np.zeros((DIM, DIM), dtype=np.float32)
    A[iu, ju] = angles
    A[ju, iu] = -angles
    return {
        "ar": A.astype(ml_dtypes.bfloat16),
        # biasq[p, t] = bias[128*t + p]
        "biasq": np.ascontiguousarray(bias.reshape(KT, P).T),
    }


def kernel(x, angles, bias, _profile=False):
    x = np.asarray(x, dtype=np.float32)
    # per-core x shards, pre-transposed to [DIM, XB] bf16 (layout only)
    xts = np.ascontiguousarray(
        x.reshape(N_CORES, XB, DIM).transpose(0, 2, 1)
    ).astype(ml_dtypes.bfloat16)
    shared = _host_inputs(angles, bias)
    nc = _get_nc()
    in_maps = [{"xt": xts[c], **shared} for c in range(N_CORES)]
    res = run_bass_kernel_spmd(
        nc, in_maps, list(range(N_CORES)), trace=bool(_profile)
    )
    _CACHE["last_result"] = res
    # device returns out.T per core; un-transpose on host (layout only)
    out = np.concatenate(
        [np.ascontiguousarray(res.results[c]["out"].T) for c in range(N_CORES)],
        axis=0,
    )
    return out


# revision 18
# speedup vs baseline: 1.1206x; 1.0640x over previous
"""Trainium2 Bass kernel for out = x @ expm(skew(angles)) + bias.

Strategy:
  - Data-parallel over the batch: x [16384, 512] is split into 8 shards of
    [2048, 512], one per NeuronCore. angles/bias are replicated.
  - Host only does layout: builds A = skew(angles) in bf16 and ships each
    core its x shard pre-transposed ([512, 2048]) in bf16 so the
    contraction dim lands on SBUF partitions. All linear algebra runs
    on-device. The output comes back transposed ([512, 2048] per core)
    and is un-transposed on host -- this puts the output *column* on the
    SBUF partition axis, which makes the bias a per-partition scalar that
    the scalar engine can add (splitting the PSUM-drain work across two
    engines; gpsimd cannot touch PSUM).
  - Rotation W = expm(A) via a degree-4 Taylor series in
    Paterson-Stockmeyer form (2 matmuls of 512^3), exploiting
    skew-symmetry so no transposes or negations are ever materialized:
        nA2 = A^T @ A              (= -A^2;  lhsT = rhs = A directly)
        A6  = -nA2 / 6             (scalar-engine scaled copy = A^2/6)
        Y   = A - nA2/4            (one DVE op per tile, = A + A^2/4)
        T   = (I + A) - nA2/2      (I built on-device via affine_select)
        F   = A6 @ Y               (A6 symmetric => its own lhsT; = A^3/6 + A^4/24)
        W   = T + F
    Truncation error ||A||^5/5! ~ 2e-4 for ||A||_2 ~ 0.48, far below the
    2e-2 gate.
  - Dtypes: all matmul operands are bf16 (the backend forbids mixing
    32-bit and 16-bit operands), streaming 1 PE cycle per output row --
    4x faster than fp32 two-pass mode -- and halving the x DMA. PSUM
    accumulation is fp32; the output is exact fp32. Measured end-to-end
    max rel err ~4e-3, dominated by the bf16 rounding of x and W.
  - Pipeline scheduling (from trace analysis): mm1 runs i-major so its 4
    PSUM groups stop staggered ~0.9us apart; the scaled copy (scalar) and
    Y/T (vector) for group i start as soon as group i stops, letting
    t-major mm2 start the moment mm1's last matmul retires. W is
    finished in 128-column quarters, emitted in the exact order the main
    loop consumes them, so the main loop starts right behind mm2.
  - PSUM budget: mm1 + main loop rotate 4 banks (eps pool), mm2 uses the
    other 4 (ops pool), so no engine ever stalls on a bank recycle.
  - Main loop: psum group (nt, mc) = out.T[128-col block nt, 512-row
    chunk mc] accumulates 4 matmuls of N=512; the PSUM->SBUF bias-add
    alternates scalar/vector per group.
"""

import numpy as np
import ml_dtypes

import concourse.bacc as bacc
import concourse.bass as bass
import concourse.mybir as mybir
import concourse.tile as tile
from concourse.bass_utils import run_bass_kernel_spmd

DIM = 512
BATCH = 16384
N_CORES = 8
XB = BATCH // N_CORES          # rows per core
P = 128                        # partitions
KT = DIM // P                  # 4 k-tiles
MT = XB // P                   # 16 m-tiles per core
XC = 4                         # m-tiles per x DMA chunk
NWARM = 30                     # clock-ramp warmup matmuls (~107ns each)
F32 = mybir.dt.float32
BF16 = mybir.dt.bfloat16

_CACHE = {}


def build_bass():
    nc = bacc.Bacc("TRN2", target_bir_lowering=False, debug=False)

    # x and A arrive pre-rearranged on host into the SBUF per-partition
    # layout, so every DMA moves one contiguous 4KB run per partition
    # (bf16 rows of the natural layout are only 1KB -> 2x packet overhead)
    xt_d = nc.dram_tensor("xt", [MT // XC, P, KT * P * XC], BF16, kind="ExternalInput")
    ar_d = nc.dram_tensor("ar", [P, KT * DIM], BF16, kind="ExternalInput")
    biasq_d = nc.dram_tensor("biasq", [P, KT], F32, kind="ExternalInput")
    out_d = nc.dram_tensor("out", [DIM, XB], F32, kind="ExternalOutput")

    AOP = mybir.AluOpType

    with tile.TileContext(nc) as tc:
        with (
            tc.tile_pool(name="const", bufs=1) as cpool,
            tc.tile_pool(name="xin", bufs=MT // XC) as xpool,
            tc.tile_pool(name="oout", bufs=6) as opool,
            tc.tile_pool(name="eps", bufs=4, space=bass.MemorySpace.PSUM) as eps,
            tc.tile_pool(name="ops", bufs=4, space=bass.MemorySpace.PSUM) as ops,
        ):
            ar_sb = cpool.tile([P, KT, DIM], BF16)     # A
            ai_sb = cpool.tile([P, KT, DIM], BF16)     # A + I
            a6_sb = cpool.tile([P, KT, DIM], BF16)     # A^2/6 (mm2 lhsT)
            y_sb = cpool.tile([P, KT, DIM], BF16)      # Y = A + A^2/4
            t_sb = cpool.tile([P, KT, DIM], F32)       # T = (I+A) + A^2/2
            m_sb = cpool.tile([P, KT, DIM], BF16)      # W
            biasq_sb = cpool.tile([P, KT], F32)
            warm_sb = cpool.tile([P, P], BF16)

            # ---- PE clock-ramp warmup: the PE p-state only ramps under
            # continuous load, and A always lands ~3-5us after the
            # prologue barrier (its DMA issue waits for the same
            # barrier), so a warmup burst bridges exactly that window ----
            nc.gpsimd.memset(warm_sb[:, :], 0.0)
            wp = ops.tile([P, DIM], F32, tag="m2", name="wp")
            for _ in range(NWARM):
                nc.tensor.matmul(
                    wp[:, 0:P], warm_sb[:, :], warm_sb[:, :], start=True, stop=True
                )

            # ---- input DMAs, priority order: A gates everything ----
            nc.sync.dma_start(
                ar_sb[:, :, :], ar_d[:, :].rearrange("p (t n) -> p t n", t=KT)
            )
            xch = []
            for c in range(MT // XC):
                xc = xpool.tile([P, KT, P * XC], BF16, tag="x")
                nc.sync.dma_start(
                    xc[:, :, :],
                    xt_d[c, :, :].rearrange("p (t m) -> p t m", t=KT),
                )
                xch.append(xc)
            nc.sync.dma_start(biasq_sb[:, :], biasq_d[:, :])

            # ---- ai = A with exact 1.0 on the diagonal (gpsimd, paced
            # only by the A-tile DMAs; global row P*t + p == col n) ----
            for t in range(KT):
                nc.gpsimd.affine_select(
                    out=ai_sb[:, t, :],
                    in_=ar_sb[:, t, :],
                    compare_op=AOP.not_equal,
                    fill=1.0,
                    base=-P * t,
                    channel_multiplier=-1,
                    pattern=[[1, DIM]],
                )

            # ---- mm1: nA2 = A^T @ A = -A^2, i-major so the 4 PSUM
            # groups stop staggered and downstream work starts early ----
            pss = [eps.tile([P, DIM], F32, tag="eps", name=f"pss{i}") for i in range(KT)]
            for i in range(KT):
                for t in range(KT):
                    nc.tensor.matmul(
                        pss[i][:, :],
                        ar_sb[:, t, P * i : P * (i + 1)],
                        ar_sb[:, t, :],
                        start=(t == 0),
                        stop=(t == KT - 1),
                    )
            # scalar engine: A6 = A^2/6 bf16 copy (the mm2 lhsT)
            for i in range(KT):
                nc.scalar.mul(a6_sb[:, i, :], pss[i][:, :], -1.0 / 6.0)
            # vector: Y_i then T_i right behind each group's stop
            for i in range(KT):
                nc.vector.scalar_tensor_tensor(
                    y_sb[:, i, :], pss[i][:, :], -0.25, ar_sb[:, i, :],
                    AOP.mult, AOP.add,
                )
                nc.vector.scalar_tensor_tensor(
                    t_sb[:, i, :], pss[i][:, :], -0.5, ai_sb[:, i, :],
                    AOP.mult, AOP.add,
                )

            # ---- mm2: F = A6 @ Y = A^3/6 + A^4/24, t-major (its deps
            # arrive in t order; starts the moment mm1 retires) ----
            # ps2[3] reuses the warmup bank (ops pool rotation; the
            # warmups retire long before mm2 reaches t=0 for group 3)
            ps2 = [ops.tile([P, DIM], F32, tag="m2", name=f"ps2{i}") for i in range(KT)]
            for t in range(KT):
                for i in range(KT):
                    nc.tensor.matmul(
                        ps2[i][:, :],
                        a6_sb[:, t, P * i : P * (i + 1)],
                        y_sb[:, t, :],
                        start=(t == 0),
                        stop=(t == KT - 1),
                    )
            # W = T + F in 128-col quarters, in main-loop consumption
            # order (q = the nt block the main loop reads as lhsT)
            for q in range(KT):
                for i in range(KT):
                    nc.vector.tensor_add(
                        m_sb[:, i, P * q : P * (q + 1)],
                        ps2[i][:, P * q : P * (q + 1)],
                        t_sb[:, i, P * q : P * (q + 1)],
                    )

            # ---- main loop: out.T = W.T @ x.T + bias ----
            # group (nt, mc): out.T rows [128*nt, 128*(nt+1)), batch cols
            # [512*mc, 512*(mc+1)); bias-add alternates scalar/vector
            for nt in range(KT):
                for mc in range(MT // XC):
                    xc = xch[mc]
                    ps = eps.tile([P, DIM], F32, tag="eps", name=f"mo{nt}_{mc}")
                    for kb in range(KT):
                        nc.tensor.matmul(
                            ps[:, :],
                            m_sb[:, kb, P * nt : P * (nt + 1)],
                            xc[:, kb, :],
                            start=(kb == 0),
                            stop=(kb == KT - 1),
                        )
                    ot = opool.tile([P, P * XC], F32, tag="o")
                    if mc % 2 == 0:
                        nc.scalar.add(ot[:, :], ps[:, :], biasq_sb[:, nt : nt + 1])
                    else:
                        nc.vector.tensor_scalar_add(
                            ot[:, :], ps[:, :], biasq_sb[:, nt : nt + 1]
                        )
                    nc.sync.dma_start(
                        out_d[P * nt : P * (nt + 1), P * XC * mc : P * XC * (mc + 1)],
                        ot[:, :],
                    )

    nc.compile()
    return nc


def _get_nc():
    if "nc" not in _CACHE:
        _CACHE["nc"] = build_bass()
    return _CACHE["nc"]


def _host_inputs(angles, bias):
    angles = np.asarray(angles, dtype=np.float32)
    bias = np.asarray(bias, dtype=np.float32)
    iu, ju = np.triu_indices(DIM, k=1)
    A = np.zeros((DIM, DIM), dtype=np.float32)
    A[iu, ju] = angles
    A[ju, iu] = -angles
    # SBUF layout: ar[p, t*512 + n] = A[128*t + p, n]
    arq = np.ascontiguousarray(
        A.reshape(KT, P, DIM).transpose(1, 0, 2).reshape(P, KT * DIM)
    )
    return {
        "ar": arq.astype(ml_dtypes.bfloat16),
        # biasq[p, t] = bias[128*t + p]
        "biasq": np.ascontiguousarray(bias.reshape(KT, P).T),
    }


def kernel(x, angles, bias, _profile=False):
    x = np.asarray(x, dtype=np.float32)
    # per-core x shards, transposed + packed into the SBUF chunk layout:
    # xts[core][c, p, t*512 + m] = x[core*XB + 512*c + m, 128*t + p]
    xts = np.ascontiguousarray(
        x.reshape(N_CORES, MT // XC, XC * P, KT, P).transpose(0, 1, 4, 3, 2)
        .reshape(N_CORES, MT // XC, P, KT * P * XC)
    ).astype(ml_dtypes.bfloat16)
    shared = _host_inputs(angles, bias)
    nc = _get_nc()
    in_maps = [{"xt": xts[c], **shared} for c in range(N_CORES)]
    res = run_bass_kernel_spmd(
        nc, in_maps, list(range(N_CORES)), trace=bool(_profile)
    )
    _CACHE["last_result"] = res
    # device returns out.T per core; un-transpose on host (layout only)
    out = np.concatenate(
        [np.ascontiguousarray(res.results[c]["out"].T) for c in range(N_CORES)],
        axis=0,
    )
    return out


# revision 22
# speedup vs baseline: 1.1648x; 1.0394x over previous
"""Trainium2 Bass kernel for out = x @ expm(skew(angles)) + bias.

Strategy:
  - Data-parallel over the batch: x [16384, 512] is split into 8 shards of
    [2048, 512], one per NeuronCore. angles/bias are replicated.
  - Host only does layout: builds A = skew(angles) in bf16 and ships each
    core its x shard pre-transposed ([512, 2048]) in bf16 so the
    contraction dim lands on SBUF partitions. All linear algebra runs
    on-device. The output comes back transposed ([512, 2048] per core)
    and is un-transposed on host -- this puts the output *column* on the
    SBUF partition axis, which makes the bias a per-partition scalar that
    the scalar engine can add (splitting the PSUM-drain work across two
    engines; gpsimd cannot touch PSUM).
  - Rotation W = expm(A) via a degree-4 Taylor series in
    Paterson-Stockmeyer form (2 matmuls of 512^3), exploiting
    skew-symmetry so no transposes or negations are ever materialized:
        nA2 = A^T @ A              (= -A^2;  lhsT = rhs = A directly)
        A6  = -nA2 / 6             (scalar-engine scaled copy = A^2/6)
        Y   = A - nA2/4            (one DVE op per tile, = A + A^2/4)
        T   = (I + A) - nA2/2      (I built on-device via affine_select)
        F   = A6 @ Y               (A6 symmetric => its own lhsT; = A^3/6 + A^4/24)
        W   = T + F
    Truncation error ||A||^5/5! ~ 2e-4 for ||A||_2 ~ 0.48, far below the
    2e-2 gate.
  - Dtypes: all matmul operands are bf16 (the backend forbids mixing
    32-bit and 16-bit operands), streaming 1 PE cycle per output row --
    4x faster than fp32 two-pass mode -- and halving the x DMA. PSUM
    accumulation is fp32; the output is exact fp32. Measured end-to-end
    max rel err ~4e-3, dominated by the bf16 rounding of x and W.
  - Pipeline scheduling (from trace analysis): mm1 runs i-major so its 4
    PSUM groups stop staggered ~0.9us apart; the scaled copy (scalar) and
    Y/T (vector) for group i start as soon as group i stops, letting
    t-major mm2 start the moment mm1's last matmul retires. W is
    finished in 128-column quarters, emitted in the exact order the main
    loop consumes them, so the main loop starts right behind mm2.
  - PSUM budget: mm1 + main loop rotate 4 banks (eps pool), mm2 uses the
    other 4 (ops pool), so no engine ever stalls on a bank recycle.
  - Main loop: psum group (nt, mc) = out.T[128-col block nt, 512-row
    chunk mc] accumulates 4 matmuls of N=512; the PSUM->SBUF bias-add
    alternates scalar/vector per group.
"""

import numpy as np
import ml_dtypes

import concourse.bacc as bacc
import concourse.bass as bass
import concourse.mybir as mybir
import concourse.tile as tile
from concourse.bass_utils import run_bass_kernel_spmd

DIM = 512
BATCH = 16384
N_CORES = 8
XB = BATCH // N_CORES          # rows per core
P = 128                        # partitions
KT = DIM // P                  # 4 k-tiles
MT = XB // P                   # 16 m-tiles per core
XC = 4                         # m-tiles per x DMA chunk
NWARM = 30                     # clock-ramp warmup matmuls (~107ns each)
F32 = mybir.dt.float32
F16 = mybir.dt.float16
BF16 = mybir.dt.bfloat16

_CACHE = {}


def build_bass():
    nc = bacc.Bacc("TRN2", target_bir_lowering=False, debug=False)

    # x and A arrive pre-rearranged on host into the SBUF per-partition
    # layout, so every DMA moves one contiguous 4KB run per partition
    # (bf16 rows of the natural layout are only 1KB -> 2x packet overhead)
    xt_d = nc.dram_tensor("xt", [MT // XC, P, KT * P * XC], BF16, kind="ExternalInput")
    ar_d = nc.dram_tensor("ar", [P, KT * DIM], BF16, kind="ExternalInput")
    biasq_d = nc.dram_tensor("biasq", [P, KT], F32, kind="ExternalInput")
    # fp16 output (exactly convertible to fp32 on host): halves the
    # out-DMA stream that dominates the kernel tail; the fp16 rounding
    # of the output adds only ~5e-4 relative error
    out_d = nc.dram_tensor("out", [DIM, XB], F16, kind="ExternalOutput")

    AOP = mybir.AluOpType

    with tile.TileContext(nc) as tc:
        with (
            tc.tile_pool(name="const", bufs=1) as cpool,
            tc.tile_pool(name="xin", bufs=MT // XC) as xpool,
            tc.tile_pool(name="oout", bufs=6) as opool,
            tc.tile_pool(name="eps", bufs=4, space=bass.MemorySpace.PSUM) as eps,
            tc.tile_pool(name="ops", bufs=4, space=bass.MemorySpace.PSUM) as ops,
        ):
            ar_sb = cpool.tile([P, KT, DIM], BF16)     # A
            ai_sb = cpool.tile([P, KT, DIM], BF16)     # A + I
            a6_sb = cpool.tile([P, KT, DIM], BF16)     # A^2/6 (mm2 lhsT)
            y_sb = cpool.tile([P, KT, DIM], BF16)      # Y = A + A^2/4
            t_sb = cpool.tile([P, KT, DIM], F32)       # T = (I+A) + A^2/2
            m_sb = cpool.tile([P, KT, DIM], BF16)      # W
            biasq_sb = cpool.tile([P, KT], F32)
            warm_sb = cpool.tile([P, P], BF16)

            # ---- PE clock-ramp warmup: the PE p-state only ramps under
            # continuous load, and A always lands ~3-5us after the
            # prologue barrier (its DMA issue waits for the same
            # barrier), so a warmup burst bridges exactly that window ----
            nc.gpsimd.memset(warm_sb[:, :], 0.0)
            wp = ops.tile([P, DIM], F32, tag="m2", name="wp")
            for _ in range(NWARM):
                nc.tensor.matmul(
                    wp[:, 0:P], warm_sb[:, :], warm_sb[:, :], start=True, stop=True
                )

            # ---- input DMAs, priority order: A gates everything ----
            nc.sync.dma_start(
                ar_sb[:, :, :], ar_d[:, :].rearrange("p (t n) -> p t n", t=KT)
            )
            xch = []
            for c in range(MT // XC):
                xc = xpool.tile([P, KT, P * XC], BF16, tag="x")
                nc.sync.dma_start(
                    xc[:, :, :],
                    xt_d[c, :, :].rearrange("p (t m) -> p t m", t=KT),
                )
                xch.append(xc)
            nc.sync.dma_start(biasq_sb[:, :], biasq_d[:, :])

            # ---- ai = A with exact 1.0 on the diagonal (gpsimd, paced
            # only by the A-tile DMAs; global row P*t + p == col n) ----
            for t in range(KT):
                nc.gpsimd.affine_select(
                    out=ai_sb[:, t, :],
                    in_=ar_sb[:, t, :],
                    compare_op=AOP.not_equal,
                    fill=1.0,
                    base=-P * t,
                    channel_multiplier=-1,
                    pattern=[[1, DIM]],
                )

            # ---- mm1: nA2 = A^T @ A = -A^2, i-major so the 4 PSUM
            # groups stop staggered and downstream work starts early ----
            pss = [eps.tile([P, DIM], F32, tag="eps", name=f"pss{i}") for i in range(KT)]
            for i in range(KT):
                for t in range(KT):
                    nc.tensor.matmul(
                        pss[i][:, :],
                        ar_sb[:, t, P * i : P * (i + 1)],
                        ar_sb[:, t, :],
                        start=(t == 0),
                        stop=(t == KT - 1),
                    )
            # scalar engine: A6 = A^2/6 bf16 copy (the mm2 lhsT)
            for i in range(KT):
                nc.scalar.mul(a6_sb[:, i, :], pss[i][:, :], -1.0 / 6.0)
            # vector: Y_i then T_i right behind each group's stop
            for i in range(KT):
                nc.vector.scalar_tensor_tensor(
                    y_sb[:, i, :], pss[i][:, :], -0.25, ar_sb[:, i, :],
                    AOP.mult, AOP.add,
                )
                nc.vector.scalar_tensor_tensor(
                    t_sb[:, i, :], pss[i][:, :], -0.5, ai_sb[:, i, :],
                    AOP.mult, AOP.add,
                )

            # ---- mm2: F = A6 @ Y = A^3/6 + A^4/24, t-major (its deps
            # arrive in t order; starts the moment mm1 retires) ----
            # ps2[3] reuses the warmup bank (ops pool rotation; the
            # warmups retire long before mm2 reaches t=0 for group 3)
            ps2 = [ops.tile([P, DIM], F32, tag="m2", name=f"ps2{i}") for i in range(KT)]
            for t in range(KT):
                for i in range(KT):
                    nc.tensor.matmul(
                        ps2[i][:, :],
                        a6_sb[:, t, P * i : P * (i + 1)],
                        y_sb[:, t, :],
                        start=(t == 0),
                        stop=(t == KT - 1),
                    )
            # W = T + F in 128-col quarters, in main-loop consumption
            # order (q = the nt block the main loop reads as lhsT)
            for q in range(KT):
                for i in range(KT):
                    nc.vector.tensor_add(
                        m_sb[:, i, P * q : P * (q + 1)],
                        ps2[i][:, P * q : P * (q + 1)],
                        t_sb[:, i, P * q : P * (q + 1)],
                    )

            # ---- main loop: out.T = W.T @ x.T + bias ----
            # group (nt, mc): out.T rows [128*nt, 128*(nt+1)), batch cols
            # [512*mc, 512*(mc+1)); bias-add alternates scalar/vector
            for nt in range(KT):
                for mc in range(MT // XC):
                    xc = xch[mc]
                    ps = eps.tile([P, DIM], F32, tag="eps", name=f"mo{nt}_{mc}")
                    for kb in range(KT):
                        nc.tensor.matmul(
                            ps[:, :],
                            m_sb[:, kb, P * nt : P * (nt + 1)],
                            xc[:, kb, :],
                            start=(kb == 0),
                            stop=(kb == KT - 1),
                        )
                    ot = opool.tile([P, P * XC], F16, tag="o")
                    if mc % 2 == 0:
                        nc.scalar.add(ot[:, :], ps[:, :], biasq_sb[:, nt : nt + 1])
                    else:
                        nc.vector.tensor_scalar_add(
                            ot[:, :], ps[:, :], biasq_sb[:, nt : nt + 1]
                        )
                    nc.sync.dma_start(
                        out_d[P * nt : P * (nt + 1), P * XC * mc : P * XC * (mc + 1)],
                        ot[:, :],
                    )

    nc.compile()
    return nc


def _get_nc():
    if "nc" not in _CACHE:
        _CACHE["nc"] = build_bass()
    return _CACHE["nc"]


def _host_inputs(angles, bias):
    angles = np.asarray(angles, dtype=np.float32)
    bias = np.asarray(bias, dtype=np.float32)
    iu, ju = np.triu_indices(DIM, k=1)
    A = np.zeros((DIM, DIM), dtype=np.float32)
    A[iu, ju] = angles
    A[ju, iu] = -angles
    # SBUF layout: ar[p, t*512 + n] = A[128*t + p, n]
    arq = np.ascontiguousarray(
        A.reshape(KT, P, DIM).transpose(1, 0, 2).reshape(P, KT * DIM)
    )
    return {
        "ar": arq.astype(ml_dtypes.bfloat16),
        # biasq[p, t] = bias[128*t + p]
        "biasq": np.ascontiguousarray(bias.reshape(KT, P).T),
    }


def kernel(x, angles, bias, _profile=False):
    x = np.asarray(x, dtype=np.float32)
    # per-core x shards, transposed + packed into the SBUF chunk layout:
    # xts[core][c, p, t*512 + m] = x[core*XB + 512*c + m, 128*t + p]
    xts = np.ascontiguousarray(
        x.reshape(N_CORES, MT // XC, XC * P, KT, P).transpose(0, 1, 4, 3, 2)
        .reshape(N_CORES, MT // XC, P, KT * P * XC)
    ).astype(ml_dtypes.bfloat16)
    shared = _host_inputs(angles, bias)
    nc = _get_nc()
    in_maps = [{"xt": xts[c], **shared} for c in range(N_CORES)]
    res = run_bass_kernel_spmd(
        nc, in_maps, list(range(N_CORES)), trace=bool(_profile)
    )
    _CACHE["last_result"] = res
    # device returns out.T per core in fp16; un-transpose and widen to
    # fp32 on host (fp16 -> fp32 is exact; layout only otherwise)
    out = np.concatenate(
        [res.results[c]["out"].T.astype(np.float32) for c in range(N_CORES)],
        axis=0,
    )
    return np.ascontiguousarray(out)


# revision 23
# speedup vs baseline: 1.2002x; 1.0304x over previous
"""Trainium2 Bass kernel for out = x @ expm(skew(angles)) + bias.

Strategy:
  - Data-parallel over the batch: x [16384, 512] is split into 8 shards of
    [2048, 512], one per NeuronCore. angles/bias are replicated.
  - Host only does layout: builds A = skew(angles) in bf16 and ships each
    core its x shard pre-transposed ([512, 2048]) in bf16 so the
    contraction dim lands on SBUF partitions. All linear algebra runs
    on-device. The output comes back transposed ([512, 2048] per core)
    and is un-transposed on host -- this puts the output *column* on the
    SBUF partition axis, which makes the bias a per-partition scalar that
    the scalar engine can add (splitting the PSUM-drain work across two
    engines; gpsimd cannot touch PSUM).
  - Rotation W = expm(A) via a degree-4 Taylor series in
    Paterson-Stockmeyer form (2 matmuls of 512^3), exploiting
    skew-symmetry so no transposes or negations are ever materialized:
        nA2 = A^T @ A              (= -A^2;  lhsT = rhs = A directly)
        A6  = -nA2 / 6             (scalar-engine scaled copy = A^2/6)
        Y   = A - nA2/4            (one DVE op per tile, = A + A^2/4)
        T   = (I + A) - nA2/2      (I built on-device via affine_select)
        F   = A6 @ Y               (A6 symmetric => its own lhsT; = A^3/6 + A^4/24)
        W   = T + F
    Truncation error ||A||^5/5! ~ 2e-4 for ||A||_2 ~ 0.48, far below the
    2e-2 gate.
  - Dtypes: all matmul operands are bf16 (the backend forbids mixing
    32-bit and 16-bit operands), streaming 1 PE cycle per output row --
    4x faster than fp32 two-pass mode -- and halving the x DMA. PSUM
    accumulation is fp32; the output is exact fp32. Measured end-to-end
    max rel err ~4e-3, dominated by the bf16 rounding of x and W.
  - Pipeline scheduling (from trace analysis): mm1 runs i-major so its 4
    PSUM groups stop staggered ~0.9us apart; the scaled copy (scalar) and
    Y/T (vector) for group i start as soon as group i stops, letting
    t-major mm2 start the moment mm1's last matmul retires. W is
    finished in 128-column quarters, emitted in the exact order the main
    loop consumes them, so the main loop starts right behind mm2.
  - PSUM budget: mm1 + main loop rotate 4 banks (eps pool), mm2 uses the
    other 4 (ops pool), so no engine ever stalls on a bank recycle.
  - Main loop: psum group (nt, mc) = out.T[128-col block nt, 512-row
    chunk mc] accumulates 4 matmuls of N=512; the PSUM->SBUF bias-add
    alternates scalar/vector per group.
"""

import numpy as np
import ml_dtypes

import concourse.bacc as bacc
import concourse.bass as bass
import concourse.mybir as mybir
import concourse.tile as tile
from concourse.bass_utils import run_bass_kernel_spmd

DIM = 512
BATCH = 16384
N_CORES = 8
XB = BATCH // N_CORES          # rows per core
P = 128                        # partitions
KT = DIM // P                  # 4 k-tiles
MT = XB // P                   # 16 m-tiles per core
XC = 4                         # m-tiles per x DMA chunk
NWARM = 30                     # clock-ramp warmup matmuls (~107ns each)
F32 = mybir.dt.float32
F16 = mybir.dt.float16
BF16 = mybir.dt.bfloat16

_CACHE = {}


def build_bass():
    nc = bacc.Bacc("TRN2", target_bir_lowering=False, debug=False)

    # x and A arrive pre-rearranged on host into the SBUF per-partition
    # layout, so every DMA moves one contiguous 4KB run per partition
    # (bf16 rows of the natural layout are only 1KB -> 2x packet overhead)
    xt_d = nc.dram_tensor("xt", [MT // XC, P, KT * P * XC], BF16, kind="ExternalInput")
    ar_d = nc.dram_tensor("ar", [P, KT * DIM], BF16, kind="ExternalInput")
    biasq_d = nc.dram_tensor("biasq", [P, KT], F32, kind="ExternalInput")
    # fp16 output (exactly convertible to fp32 on host): halves the
    # out-DMA stream that dominates the kernel tail; the fp16 rounding
    # of the output adds only ~5e-4 relative error
    out_d = nc.dram_tensor("out", [DIM, XB], F16, kind="ExternalOutput")

    AOP = mybir.AluOpType

    with tile.TileContext(nc) as tc:
        with (
            tc.tile_pool(name="const", bufs=1) as cpool,
            tc.tile_pool(name="xin", bufs=MT // XC) as xpool,
            tc.tile_pool(name="oout", bufs=6) as opool,
            tc.tile_pool(name="eps", bufs=4, space=bass.MemorySpace.PSUM) as eps,
            tc.tile_pool(name="ops", bufs=4, space=bass.MemorySpace.PSUM) as ops,
        ):
            ar_sb = cpool.tile([P, KT, DIM], BF16)     # A
            ai_sb = cpool.tile([P, KT, DIM], BF16)     # A + I
            a6_sb = cpool.tile([P, KT, DIM], BF16)     # A^2/6 (mm2 lhsT)
            y_sb = cpool.tile([P, KT, DIM], BF16)      # Y = A + A^2/4
            t_sb = cpool.tile([P, KT, DIM], F32)       # T = (I+A) + A^2/2
            m_sb = cpool.tile([P, KT, DIM], BF16)      # W
            biasq_sb = cpool.tile([P, KT], F32)
            warm_sb = cpool.tile([P, P], BF16)

            # ---- PE clock-ramp warmup: the PE p-state only ramps under
            # continuous load, and A always lands ~3-5us after the
            # prologue barrier (its DMA issue waits for the same
            # barrier), so a warmup burst bridges exactly that window ----
            nc.gpsimd.memset(warm_sb[:, :], 0.0)
            wp = ops.tile([P, DIM], F32, tag="m2", name="wp")
            for _ in range(NWARM):
                nc.tensor.matmul(
                    wp[:, 0:P], warm_sb[:, :], warm_sb[:, :], start=True, stop=True
                )

            # ---- input DMAs. A gates everything, so it gets the DMA
            # engines to itself: the x loads are held back (via tiny
            # memset WAW deps on gpsimd, released just after A lands)
            # so their 2MB doesn't steal A's bandwidth. x still lands
            # ~5us before the main loop needs it. ----
            nc.sync.dma_start(
                ar_sb[:, :, :], ar_d[:, :].rearrange("p (t n) -> p t n", t=KT)
            )
            nc.sync.dma_start(biasq_sb[:, :], biasq_d[:, :])
            xch = [
                xpool.tile([P, KT, P * XC], BF16, tag="x", name=f"xc{c}")
                for c in range(MT // XC)
            ]

            def ai_select(t):
                # ai = A with exact 1.0 on the diagonal
                # (global row P*t + p == col n)
                nc.gpsimd.affine_select(
                    out=ai_sb[:, t, :],
                    in_=ar_sb[:, t, :],
                    compare_op=AOP.not_equal,
                    fill=1.0,
                    base=-P * t,
                    channel_multiplier=-1,
                    pattern=[[1, DIM]],
                )

            ai_select(0)
            for c in range(MT // XC):
                nc.gpsimd.memset(xch[c][0:1, 0, 0:1], 0.0)
            for t in range(1, KT):
                ai_select(t)
            for c in range(MT // XC):
                nc.sync.dma_start(
                    xch[c][:, :, :],
                    xt_d[c, :, :].rearrange("p (t m) -> p t m", t=KT),
                )

            # ---- mm1: nA2 = A^T @ A = -A^2, i-major so the 4 PSUM
            # groups stop staggered and downstream work starts early ----
            pss = [eps.tile([P, DIM], F32, tag="eps", name=f"pss{i}") for i in range(KT)]
            for i in range(KT):
                for t in range(KT):
                    nc.tensor.matmul(
                        pss[i][:, :],
                        ar_sb[:, t, P * i : P * (i + 1)],
                        ar_sb[:, t, :],
                        start=(t == 0),
                        stop=(t == KT - 1),
                    )
            # scalar engine: A6 = A^2/6 bf16 copy (the mm2 lhsT)
            for i in range(KT):
                nc.scalar.mul(a6_sb[:, i, :], pss[i][:, :], -1.0 / 6.0)
            # vector: Y_i then T_i right behind each group's stop
            for i in range(KT):
                nc.vector.scalar_tensor_tensor(
                    y_sb[:, i, :], pss[i][:, :], -0.25, ar_sb[:, i, :],
                    AOP.mult, AOP.add,
                )
                nc.vector.scalar_tensor_tensor(
                    t_sb[:, i, :], pss[i][:, :], -0.5, ai_sb[:, i, :],
                    AOP.mult, AOP.add,
                )

            # ---- mm2: F = A6 @ Y = A^3/6 + A^4/24, t-major (its deps
            # arrive in t order; starts the moment mm1 retires) ----
            # ps2[3] reuses the warmup bank (ops pool rotation; the
            # warmups retire long before mm2 reaches t=0 for group 3)
            ps2 = [ops.tile([P, DIM], F32, tag="m2", name=f"ps2{i}") for i in range(KT)]
            for t in range(KT):
                for i in range(KT):
                    nc.tensor.matmul(
                        ps2[i][:, :],
                        a6_sb[:, t, P * i : P * (i + 1)],
                        y_sb[:, t, :],
                        start=(t == 0),
                        stop=(t == KT - 1),
                    )
            # W = T + F in 128-col quarters, in main-loop consumption
            # order (q = the nt block the main loop reads as lhsT)
            for q in range(KT):
                for i in range(KT):
                    nc.vector.tensor_add(
                        m_sb[:, i, P * q : P * (q + 1)],
                        ps2[i][:, P * q : P * (q + 1)],
                        t_sb[:, i, P * q : P * (q + 1)],
                    )

            # ---- main loop: out.T = W.T @ x.T + bias ----
            # group (nt, mc): out.T rows [128*nt, 128*(nt+1)), batch cols
            # [512*mc, 512*(mc+1)); bias-add alternates scalar/vector
            for nt in range(KT):
                for mc in range(MT // XC):
                    xc = xch[mc]
                    ps = eps.tile([P, DIM], F32, tag="eps", name=f"mo{nt}_{mc}")
                    for kb in range(KT):
                        nc.tensor.matmul(
                            ps[:, :],
                            m_sb[:, kb, P * nt : P * (nt + 1)],
                            xc[:, kb, :],
                            start=(kb == 0),
                            stop=(kb == KT - 1),
                        )
                    ot = opool.tile([P, P * XC], F16, tag="o")
                    if mc % 2 == 0:
                        nc.scalar.add(ot[:, :], ps[:, :], biasq_sb[:, nt : nt + 1])
                    else:
                        nc.vector.tensor_scalar_add(
                            ot[:, :], ps[:, :], biasq_sb[:, nt : nt + 1]
                        )
                    nc.sync.dma_start(
                        out_d[P * nt : P * (nt + 1), P * XC * mc : P * XC * (mc + 1)],
                        ot[:, :],
                    )

    nc.compile()
    return nc


def _get_nc():
    if "nc" not in _CACHE:
        _CACHE["nc"] = build_bass()
    return _CACHE["nc"]


def _host_inputs(angles, bias):
    angles = np.asarray(angles, dtype=np.float32)
    bias = np.asarray(bias, dtype=np.float32)
    iu, ju = np.triu_indices(DIM, k=1)
    A = np.zeros((DIM, DIM), dtype=np.float32)
    A[iu, ju] = angles
    A[ju, iu] = -angles
    # SBUF layout: ar[p, t*512 + n] = A[128*t + p, n]
    arq = np.ascontiguousarray(
        A.reshape(KT, P, DIM).transpose(1, 0, 2).reshape(P, KT * DIM)
    )
    return {
        "ar": arq.astype(ml_dtypes.bfloat16),
        # biasq[p, t] = bias[128*t + p]
        "biasq": np.ascontiguousarray(bias.reshape(KT, P).T),
    }


def kernel(x, angles, bias, _profile=False):
    x = np.asarray(x, dtype=np.float32)
    # per-core x shards, transposed + packed into the SBUF chunk layout:
    # xts[core][c, p, t*512 + m] = x[core*XB + 512*c + m, 128*t + p]
    xts = np.ascontiguousarray(
        x.reshape(N_CORES, MT // XC, XC * P, KT, P).transpose(0, 1, 4, 3, 2)
        .reshape(N_CORES, MT // XC, P, KT * P * XC)
    ).astype(ml_dtypes.bfloat16)
    shared = _host_inputs(angles, bias)
    nc = _get_nc()
    in_maps = [{"xt": xts[c], **shared} for c in range(N_CORES)]
    res = run_bass_kernel_spmd(
        nc, in_maps, list(range(N_CORES)), trace=bool(_profile)
    )
    _CACHE["last_result"] = res
    # device returns out.T per core in fp16; un-transpose and widen to
    # fp32 on host (fp16 -> fp32 is exact; layout only otherwise)
    out = np.concatenate(
        [res.results[c]["out"].T.astype(np.float32) for c in range(N_CORES)],
        axis=0,
    )
    return np.ascontiguousarray(out)
